# revision 31
# baseline (speedup 1.0000x reference)
"""Trainium2 Bass kernel for the LiquidNeuralNetwork problem.

Math: h' = -alpha*h + beta*tanh(x_t @ W_fc.T + b_fc + gamma*h), piecewise-
constant input over 64 intervals; output = h(1.0) @ W_out.T + b_out.

Each hidden unit's ODE is independent given u = x@W_fc.T, so we shard the
HIDDEN dim across the 8 cores (128 units each, batch of 256 as the free
dim). Per core (variant "split_noyt", the default):
  1. U-matmul (fp16 operands, fp32 PSUM): u[h, (s,b)] for its hidden slice,
     trickled in 512-col blocks between recurrence steps so the PE never
     takes a long burst on the critical path.
  2. 64 sequential exponential-midpoint steps on the y = gamma*h state:
       y' = E*y + D*tanh(u + y + b_fc),  E = exp(-alpha*dt) etc.
     The batch is SPLIT INTO TWO INDEPENDENT 128-col CHAINS so the scalar
     engine's tanh of one half overlaps the tensor engine's matmuls of the
     other (the act->matmul->act dependency chain, not engine throughput, is
     the bottleneck). Per-unit scales are applied as diagonal matmuls
     accumulating in PSUM (fp32 for the state term - f32r is no faster at
     128 moving cols - fp16 for activation/input terms); tanh runs on the
     scalar engine reading PSUM directly with b_fc as per-partition bias.
     The state path (py = dE*y + dD*a2 -> DVE copy) is kept bit-identical
     to the proven single-chain baseline: the ODE is chaotic for high
     |gamma*beta| units (~1e3-1e4 amplification of state rounding), so
     alternative state arithmetic (e.g. y = pq - u) re-rolls the HW error
     draw from 1.4e-3 to ~9e-3 against the 2e-2 gate.
  3. Readout matmul against W_out.T / gamma (gamma folded in on host).
Host sums the 8 partial readouts and adds b_out.

Timing methodology: the axon/PJRT tunnel adds ms-scale per-dispatch overhead
(a trivial copy NEFF measures 3-15 ms/dispatch), so the whole kernel body is
also compiled wrapped in a For_i hardware loop (K complete passes per
dispatch, each re-reading x/weights from HBM and re-writing the output), and
HW exec time is the dispatch-time difference (T(K)-T(1))/(K-1), which
cancels dispatch overhead exactly. Measured: 166 us/pass vs 186 us for the
single-chain baseline and 13.1 ms for the per-dispatch wall-clock metric.
"""
import sys

sys.path.insert(0, "/opt/trn_rl_repo")

import numpy as np

import concourse.bacc as bacc
import concourse.mybir as mybir
import concourse.tile as tile
from concourse.bass_utils import run_bass_kernel_spmd

B, S, I, H, O = 256, 64, 256, 1024, 10
NCORES = 8
HS = H // NCORES          # hidden rows per core
DT = 1.0 / S
COLS = B                  # free dim of the recurrence state
NB = 512                  # matmul moving-dim block
CHUNK = 2048              # x/u column chunk (8 intervals worth)
NCHUNKS = S * B // CHUNK  # 8

F32 = mybir.dt.float32
BF16 = mybir.dt.bfloat16
F16 = mybir.dt.float16
F32R = mybir.dt.float32r
TANH = mybir.ActivationFunctionType.Tanh

_built = {}


def _build_nc_split(k_loop=1, use_ysub=True, sched=0):
    """Optimized per-core NEFF: the batch (256 cols) is split into two
    independent 128-col recurrence chains so the scalar engine's tanh of one
    half overlaps the tensor engine's matmuls of the other. The py group and
    state copy of the baseline are replaced by the exact identity
    y_{s+1} = pq_{s+1} - u_{s+1} (one DVE op), since
    pq_{s+1} = E*y_s + u_{s+1} + D*a2 and y_{s+1} = E*y_s + D*a2.
    """
    key = ("split", k_loop, use_ysub, sched)
    if key in _built:
        return _built[key]
    nc = bacc.Bacc("TRN2", target_bir_lowering=False, debug=False,
                   num_devices=NCORES)

    HB = COLS // 2  # 128-col half-batch per chain

    xk = [nc.dram_tensor(f"x{k}", [128, S * B], F16, kind="ExternalInput")
          for k in range(2)]
    wk = [nc.dram_tensor(f"w{k}", [128, HS], F16, kind="ExternalInput")
          for k in range(2)]
    dEh_d = nc.dram_tensor("dEh", [HS, HS], F32, kind="ExternalInput")
    dE_d = nc.dram_tensor("dE", [HS, HS], F32, kind="ExternalInput")
    dDh_d = nc.dram_tensor("dDh", [HS, HS], F16, kind="ExternalInput")
    dD_d = nc.dram_tensor("dD", [HS, HS], F16, kind="ExternalInput")
    dI_d = nc.dram_tensor("dI", [HS, HS], F16, kind="ExternalInput")
    bfc_d = nc.dram_tensor("bfc", [HS, 1], F32, kind="ExternalInput")
    wo_d = nc.dram_tensor("wo", [HS, O], F32, kind="ExternalInput")
    out_d = nc.dram_tensor("out", [O, B], F32, kind="ExternalOutput")

    with tile.TileContext(nc) as tc:
        with tc.tile_pool(name="const", bufs=1) as cpool, \
             tc.tile_pool(name="xpool", bufs=1) as xpool, \
             tc.tile_pool(name="upool", bufs=1) as upool, \
             tc.tile_pool(name="state", bufs=2) as spool, \
             tc.tile_pool(name="act", bufs=3) as apool, \
             tc.tile_pool(name="psmp", bufs=1, space="PSUM") as psmp, \
             tc.tile_pool(name="psqp", bufs=2, space="PSUM") as psqp, \
             tc.tile_pool(name="psu", bufs=2, space="PSUM") as psu:
            psm = [psmp, psmp]
            psq = [psqp, psqp]

            def body():
                wt = []
                for k in range(2):
                    t = cpool.tile([128, HS], F16, tag=f"w{k}", name=f"w{k}s")
                    nc.sync.dma_start(t[:], wk[k][:])
                    wt.append(t)
                dEh = cpool.tile([HS, HS], F32, tag="dEh")
                dE = cpool.tile([HS, HS], F32, tag="dE")
                dDh = cpool.tile([HS, HS], F16, tag="dDh")
                dD = cpool.tile([HS, HS], F16, tag="dD")
                dI = cpool.tile([HS, HS], F16, tag="dI")
                bfc = cpool.tile([HS, 1], F32, tag="bfc")
                wo = cpool.tile([HS, O], F32, tag="wo")
                for t, d in [(dDh, dDh_d), (dD, dD_d), (dI, dI_d),
                             (bfc, bfc_d), (wo, wo_d), (dEh, dEh_d),
                             (dE, dE_d)]:
                    nc.sync.dma_start(t[:], d[:])

                xt = [[None] * NCHUNKS for _ in range(2)]
                for k in range(2):
                    for c in range(NCHUNKS):
                        t = xpool.tile([128, CHUNK], F16, tag=f"x{k}_{c}",
                                       name=f"x{k}_{c}s")
                        nc.sync.dma_start(t[:], xk[k][:, c * CHUNK:(c + 1) * CHUNK])
                        xt[k][c] = t

                ut = [upool.tile([128, CHUNK], F16, tag=f"u{c}", name=f"u{c}s")
                      for c in range(NCHUNKS)]

                def emit_u_block(c, nb):
                    pu = psu.tile([128, NB], F32, tag="pu", name="pu")
                    sl = slice(nb * NB, (nb + 1) * NB)
                    nc.tensor.matmul(pu[:], wt[0][:], xt[0][c][:, sl],
                                     start=True, stop=False)
                    nc.tensor.matmul(pu[:], wt[1][:], xt[1][c][:, sl],
                                     start=False, stop=True)
                    nc.vector.tensor_copy(ut[c][:, sl], pu[:])

                def emit_u_chunk(c):
                    for nb in range(CHUNK // NB):
                        emit_u_block(c, nb)

                # u chunks 0,1 up front; later blocks trickled per step pair
                emit_u_chunk(0)
                emit_u_chunk(1)

                def u_sl(s, h):
                    c, off = divmod(s * COLS, CHUNK)
                    off += h * HB
                    return ut[c][:, off:off + HB]

                # per-half state + initial pq = u_0 (y=0)
                y = [None, None]
                pq = [None, None]
                a1 = [None, None]
                for h in range(2):
                    y[h] = spool.tile([HS, HB], F32, tag=f"y{h}",
                                      name=f"y{h}")
                    nc.vector.memset(y[h][:], 0.0)
                    pq[h] = psq[h].tile([HS, HB], F32, tag=f"pq{h}",
                                        name=f"pq{h}")
                    nc.tensor.matmul(pq[h][:], dI[:], u_sl(0, h),
                                     start=True, stop=True)

                for s in range(S):
                    # trickle u-matmul blocks for chunk c+2 (4 blocks over
                    # the 8 steps a chunk lasts -> one block every 2 steps)
                    c = (s * COLS) // CHUNK
                    phase = s % (CHUNK // COLS)
                    if phase % 2 == 0 and c + 2 < NCHUNKS:
                        emit_u_block(c + 2, phase // 2)

                    pm = [None, None]
                    if sched == 1:
                        # issue act-independent pm matmuls before the a1
                        # activations so PE overlaps ACT
                        for h in range(2):
                            pm[h] = psm[h].tile([HS, HB], F32, tag=f"pm{h}",
                                                name=f"pm{h}")
                            nc.tensor.matmul(pm[h][:], dEh[:], y[h][:],
                                             start=True, stop=False)
                            nc.tensor.matmul(pm[h][:], dI[:], u_sl(s, h),
                                             start=False, stop=False)
                    for h in range(2):
                        a1[h] = apool.tile([HS, HB], F16, tag=f"a1{h}",
                                           name=f"a1{h}")
                        nc.scalar.activation(a1[h][:], pq[h][:], TANH,
                                             bias=bfc[:], scale=1.0)

                    if sched == 0:
                        for h in range(2):
                            pm[h] = psm[h].tile([HS, HB], F32, tag=f"pm{h}",
                                                name=f"pm{h}")
                            nc.tensor.matmul(pm[h][:], dEh[:], y[h][:],
                                             start=True, stop=False)
                            nc.tensor.matmul(pm[h][:], dI[:], u_sl(s, h),
                                             start=False, stop=False)
                    for h in range(2):
                        nc.tensor.matmul(pm[h][:], dDh[:], a1[h][:],
                                         start=False, stop=True)

                    pqn = [None, None]

                    def emit_pq_state():
                        for h in range(2):
                            pqn[h] = psq[h].tile([HS, HB], F32, tag=f"pq{h}",
                                                 name=f"pq{h}")
                            nc.tensor.matmul(pqn[h][:], dE[:], y[h][:],
                                             start=True, stop=False)
                            if use_ysub != "acc":
                                nc.tensor.matmul(pqn[h][:], dI[:],
                                                 u_sl(s + 1, h),
                                                 start=False, stop=False)

                    if (sched == 1 or use_ysub == "acc") and s < S - 1:
                        # pq/py state matmuls run on PE during the a2 tanh
                        emit_pq_state()

                    a2 = [None, None]
                    for h in range(2):
                        a2[h] = apool.tile([HS, HB], F16, tag=f"a2{h}",
                                           name=f"a2{h}")
                        nc.scalar.activation(a2[h][:], pm[h][:], TANH,
                                             bias=bfc[:], scale=1.0)

                    if s < S - 1:
                        if sched == 0 and use_ysub != "acc":
                            emit_pq_state()
                        yn = [None, None]
                        if use_ysub == "acc":
                            # py = E*y + D*a2 (exact baseline state path);
                            # DVE-copy y out; then accumulate u' into the
                            # same bank so a1' reads (py + u') with no
                            # separate pq group
                            for h in range(2):
                                nc.tensor.matmul(pqn[h][:], dD[:], a2[h][:],
                                                 start=False, stop=True)
                            for h in range(2):
                                yn[h] = spool.tile([HS, HB], F32,
                                                   tag=f"y{h}", name=f"y{h}")
                                nc.vector.tensor_copy(yn[h][:], pqn[h][:])
                            for h in range(2):
                                nc.tensor.matmul(pqn[h][:], dI[:],
                                                 u_sl(s + 1, h),
                                                 start=False, stop=True)
                            pq = pqn
                            y = yn
                            continue
                        for h in range(2):
                            nc.tensor.matmul(pqn[h][:], dD[:], a2[h][:],
                                             start=False, stop=True)
                        if use_ysub:
                            # y_{s+1} = pq_{s+1} - u_{s+1}  (exact)
                            for h in range(2):
                                yn[h] = spool.tile([HS, HB], F32,
                                                   tag=f"y{h}", name=f"y{h}")
                                nc.vector.tensor_tensor(
                                    yn[h][:], pqn[h][:], u_sl(s + 1, h),
                                    mybir.AluOpType.subtract)
                        else:
                            # A/B check: baseline-style py group + copy
                            for h in range(2):
                                pyl = psm[h].tile([HS, HB], F32,
                                                  tag=f"pm{h}",
                                                  name=f"py{h}_{s}")
                                nc.tensor.matmul(pyl[:], dE[:], y[h][:],
                                                 start=True, stop=False)
                                nc.tensor.matmul(pyl[:], dD[:], a2[h][:],
                                                 start=False, stop=True)
                                yn[h] = spool.tile([HS, HB], F32,
                                                   tag=f"y{h}", name=f"y{h}")
                                nc.vector.tensor_copy(yn[h][:], pyl[:])
                        pq = pqn
                        y = yn
                    else:
                        # last step: y_S = E*y + D*a2 directly
                        pyl = [None, None]
                        for h in range(2):
                            pyl[h] = psm[h].tile([HS, HB], F32, tag=f"pm{h}",
                                                 name=f"py{h}")
                            nc.tensor.matmul(pyl[h][:], dE[:], y[h][:],
                                             start=True, stop=False)
                            nc.tensor.matmul(pyl[h][:], dD[:], a2[h][:],
                                             start=False, stop=True)
                        yn = [None, None]
                        for h in range(2):
                            yn[h] = spool.tile([HS, HB], F32, tag=f"y{h}",
                                               name=f"y{h}")
                            nc.vector.tensor_copy(yn[h][:], pyl[h][:])
                        y = yn

                # readout per half: out[o, b] = sum_p wo[p, o] * y[p, b]
                o32 = spool.tile([O, COLS], F32, tag="o32", name="o32")
                for h in range(2):
                    po = psm[h].tile([O, HB], F32, tag=f"pm{h}", name=f"po{h}")
                    nc.tensor.matmul(po[:], wo[:], y[h][:], start=True,
                                     stop=True)
                    nc.vector.tensor_copy(o32[:, h * HB:(h + 1) * HB], po[:])
                nc.sync.dma_start(out_d[:], o32[:])

            if k_loop == 1:
                body()
            else:
                with tc.For_i(0, k_loop):
                    body()

    nc.compile()
    _built[key] = nc
    return nc


def _build_nc(use_f32r=False, k_loop=1):
    """Build the per-core NEFF. k_loop>1 wraps the ENTIRE kernel (x DMA,
    u-matmul, recurrence, readout, output DMA) in a For_i hardware loop so
    steady-state HW time per pass can be measured without per-dispatch
    overhead."""
    key = (use_f32r, k_loop)
    if key in _built:
        return _built[key]
    nc = bacc.Bacc("TRN2", target_bir_lowering=False, debug=False,
                   num_devices=NCORES)

    # typed f32r: declare the fp32 state-matmul operands (dEh/dE/wo and the
    # recurrence state y) as float32r end-to-end so the BIR verifier sees
    # f32r-producing instructions feeding the f32r matmuls
    FSR = F32R if use_f32r else F32

    xk = [nc.dram_tensor(f"x{k}", [128, S * B], F16, kind="ExternalInput")
          for k in range(2)]
    wk = [nc.dram_tensor(f"w{k}", [128, HS], F16, kind="ExternalInput")
          for k in range(2)]
    dEh_d = nc.dram_tensor("dEh", [HS, HS], FSR, kind="ExternalInput")
    dE_d = nc.dram_tensor("dE", [HS, HS], FSR, kind="ExternalInput")
    dDh_d = nc.dram_tensor("dDh", [HS, HS], F16, kind="ExternalInput")
    dD_d = nc.dram_tensor("dD", [HS, HS], F16, kind="ExternalInput")
    dI_d = nc.dram_tensor("dI", [HS, HS], F16, kind="ExternalInput")
    bfc_d = nc.dram_tensor("bfc", [HS, 1], F32, kind="ExternalInput")
    wo_d = nc.dram_tensor("wo", [HS, O], FSR, kind="ExternalInput")
    out_d = nc.dram_tensor("out", [O, B], F32, kind="ExternalOutput")

    with tile.TileContext(nc) as tc:
        with tc.tile_pool(name="const", bufs=1) as cpool, \
             tc.tile_pool(name="xpool", bufs=1) as xpool, \
             tc.tile_pool(name="upool", bufs=1) as upool, \
             tc.tile_pool(name="state", bufs=3) as spool, \
             tc.tile_pool(name="act", bufs=3) as apool, \
             tc.tile_pool(name="psq", bufs=2, space="PSUM") as psq, \
             tc.tile_pool(name="psm", bufs=2, space="PSUM") as psm, \
             tc.tile_pool(name="psy", bufs=2, space="PSUM") as psy, \
             tc.tile_pool(name="psu", bufs=2, space="PSUM") as psu:

            def body():
                # ---- constants into SBUF ----
                wt = []
                for k in range(2):
                    t = cpool.tile([128, HS], F16, tag=f"w{k}", name=f"w{k}s")
                    nc.sync.dma_start(t[:], wk[k][:])
                    wt.append(t)
                dEh = cpool.tile([HS, HS], FSR, tag="dEh")
                dE = cpool.tile([HS, HS], FSR, tag="dE")
                dDh = cpool.tile([HS, HS], F16, tag="dDh")
                dD = cpool.tile([HS, HS], F16, tag="dD")
                dI = cpool.tile([HS, HS], F16, tag="dI")
                bfc = cpool.tile([HS, 1], F32, tag="bfc")
                wo = cpool.tile([HS, O], FSR, tag="wo")
                pairs = [(dDh, dDh_d), (dD, dD_d), (dI, dI_d), (bfc, bfc_d),
                         (wo, wo_d), (dEh, dEh_d), (dE, dE_d)]
                for t, d in pairs:
                    nc.sync.dma_start(t[:], d[:])

                # ---- x chunks ----
                xt = [[None] * NCHUNKS for _ in range(2)]
                for k in range(2):
                    for c in range(NCHUNKS):
                        t = xpool.tile([128, CHUNK], F16, tag=f"x{k}_{c}",
                                       name=f"x{k}_{c}s")
                        nc.sync.dma_start(t[:], xk[k][:, c * CHUNK:(c + 1) * CHUNK])
                        xt[k][c] = t

                ut = [upool.tile([128, CHUNK], F16, tag=f"u{c}", name=f"u{c}s")
                      for c in range(NCHUNKS)]

                def emit_u_chunk(c):
                    for nb in range(CHUNK // NB):
                        pu = psu.tile([128, NB], F32, tag="pu", name="pu")
                        sl = slice(nb * NB, (nb + 1) * NB)
                        nc.tensor.matmul(pu[:], wt[0][:], xt[0][c][:, sl],
                                         start=True, stop=False)
                        nc.tensor.matmul(pu[:], wt[1][:], xt[1][c][:, sl],
                                         start=False, stop=True)
                        nc.vector.tensor_copy(ut[c][:, sl], pu[:])

                def mmr(ps, lhs32, rhs32, start, stop):
                    # fp32 state-term matmul (operands FSR-typed: f32r full
                    # rate at >=256 moving cols, or plain fp32 at 1/4 rate)
                    nc.tensor.matmul(ps, lhs32, rhs32, start=start, stop=stop)

                # u chunks 0,1 up front; c+1 emitted while chunk c recurs
                emit_u_chunk(0)
                emit_u_chunk(1)

                y = spool.tile([HS, COLS], FSR, tag="y")
                if use_f32r:
                    # memset can't write f32r directly (ISA check); zero an
                    # fp32 scratch and convert via DVE copy
                    z32 = spool.tile([HS, COLS], F32, tag="z32", name="z32")
                    nc.vector.memset(z32[:], 0.0)
                    nc.vector.tensor_copy(y[:], z32[:])
                else:
                    nc.vector.memset(y[:], 0.0)
                pq = psq.tile([HS, COLS], F32, tag="pq", name="pq")
                nc.tensor.matmul(pq[:], dI[:], ut[0][:, 0:COLS],
                                 start=True, stop=True)

                for s in range(S):
                    c, off = divmod(s * COLS, CHUNK)
                    if s % (CHUNK // COLS) == 0 and c + 2 < NCHUNKS:
                        emit_u_chunk(c + 2)
                    u_s = ut[c][:, off:off + COLS]

                    a1 = apool.tile([HS, COLS], F16, tag="a1", name="a1")
                    nc.scalar.activation(a1[:], pq[:], TANH, bias=bfc[:],
                                         scale=1.0)

                    pm = psm.tile([HS, COLS], F32, tag="pm", name="pm")
                    mmr(pm[:], dEh[:], y[:], True, False)
                    nc.tensor.matmul(pm[:], dI[:], u_s, start=False, stop=False)
                    nc.tensor.matmul(pm[:], dDh[:], a1[:], start=False,
                                     stop=True)

                    a2 = apool.tile([HS, COLS], F16, tag="a2", name="a2")
                    nc.scalar.activation(a2[:], pm[:], TANH, bias=bfc[:],
                                         scale=1.0)

                    if s < S - 1:
                        cn, offn = divmod((s + 1) * COLS, CHUNK)
                        u_n = ut[cn][:, offn:offn + COLS]
                        pq = psq.tile([HS, COLS], F32, tag="pq", name="pq")
                        mmr(pq[:], dE[:], y[:], True, False)
                        nc.tensor.matmul(pq[:], dI[:], u_n, start=False,
                                         stop=False)
                        nc.tensor.matmul(pq[:], dD[:], a2[:], start=False,
                                         stop=True)

                    py = psy.tile([HS, COLS], F32, tag="py", name="py")
                    mmr(py[:], dE[:], y[:], True, False)
                    nc.tensor.matmul(py[:], dD[:], a2[:], start=False,
                                     stop=True)
                    y = spool.tile([HS, COLS], FSR, tag="y", name="y")
                    nc.vector.tensor_copy(y[:], py[:])

                # readout: out[o, b] = sum_p wo[p, o] * y64[p, b]
                po = psm.tile([O, COLS], F32, tag="pm", name="po")
                mmr(po[:], wo[:], y[:], True, True)
                o32 = spool.tile([O, COLS], F32, tag="o32", name="o32")
                nc.vector.tensor_copy(o32[:], po[:])
                nc.sync.dma_start(out_d[:], o32[:])

            if k_loop == 1:
                body()
            else:
                with tc.For_i(0, k_loop):
                    body()

    nc.compile()
    _built[key] = nc
    return nc


def _phi1(a, t):
    z = a * t
    small = np.abs(z) < 1e-6
    return np.where(small, 1 - z / 2 + z * z / 6,
                    (1 - np.exp(-z)) / np.where(small, 1, z))


def _in_maps(x, W_fc, b_fc, alpha, beta, gamma, W_out):
    a64 = alpha.astype(np.float64)
    b64 = beta.astype(np.float64)
    g64 = gamma.astype(np.float64)
    Eh = np.exp(-a64 * DT / 2)
    E = np.exp(-a64 * DT)
    Dh = g64 * b64 * (DT / 2) * _phi1(a64, DT / 2)
    D = g64 * b64 * DT * _phi1(a64, DT)

    # xT[i, s*B + b] = x[b, s, i]
    xT = np.ascontiguousarray(x.transpose(2, 1, 0).reshape(I, S * B))
    x16 = xT.astype(np.float16)
    eye16 = np.eye(HS, dtype=np.float16)

    g_safe = np.where(np.abs(g64) < 1e-30, 1e-30, g64)
    maps = []
    for c in range(NCORES):
        sl = slice(c * HS, (c + 1) * HS)
        wT = np.ascontiguousarray(W_fc[sl, :].T.astype(np.float16))  # [I, HS]
        maps.append({
            "x0": x16[:128], "x1": x16[128:],
            "w0": np.ascontiguousarray(wT[:128]),
            "w1": np.ascontiguousarray(wT[128:]),
            "dEh": np.ascontiguousarray(np.diag(Eh[sl]).astype(np.float32)),
            "dE": np.ascontiguousarray(np.diag(E[sl]).astype(np.float32)),
            "dDh": np.ascontiguousarray(np.diag(Dh[sl]).astype(np.float16)),
            "dD": np.ascontiguousarray(np.diag(D[sl]).astype(np.float16)),
            "dI": eye16,
            "bfc": b_fc[sl].astype(np.float32).reshape(HS, 1),
            "wo": np.ascontiguousarray(
                (W_out.astype(np.float64)[:, sl] / g_safe[sl][None, :])
                .T.astype(np.float32)),
        })
    return maps


def _make_runner(nc, maps):
    """Build a jitted sharded executor over the 8 cores for a compiled nc.
    Returns (run_once, outs0) where run_once(prev_outs) executes one dispatch
    with device-resident inputs, donating prev_outs as output buffers."""
    import jax
    from jax.sharding import Mesh, PartitionSpec
    from jax.experimental.shard_map import shard_map
    from concourse import bass2jax as b2j
    import concourse.mybir as mb

    b2j.install_neuronx_cc_hook()
    partition_name = (nc.partition_id_tensor.name
                      if nc.partition_id_tensor else None)
    in_names, out_names, out_avals, zero_outs = [], [], [], []
    for alloc in nc.m.functions[0].allocations:
        if not isinstance(alloc, mb.MemoryLocationSet):
            continue
        name = alloc.memorylocations[0].name
        if alloc.kind == "ExternalInput":
            if name != partition_name:
                in_names.append(name)
        elif alloc.kind == "ExternalOutput":
            shape = tuple(alloc.tensor_shape)
            dtype = mb.dt.np(alloc.dtype)
            out_avals.append(jax.core.ShapedArray(shape, dtype))
            zero_outs.append(np.zeros(shape, dtype))
            out_names.append(name)
    n_params = len(in_names)
    n_outs = len(out_avals)
    in_names.extend(out_names)
    if partition_name is not None:
        in_names.append(partition_name)

    donate = tuple(range(n_params, n_params + n_outs))

    def _body(*args):
        operands = list(args)
        if partition_name is not None:
            operands.append(b2j.partition_id_tensor())
        outs = b2j._bass_exec_p.bind(
            *operands, out_avals=tuple(out_avals), in_names=tuple(in_names),
            out_names=tuple(out_names), lowering_input_output_aliases=(),
            sim_require_finite=True, sim_require_nnan=True, nc=nc)
        return tuple(outs)

    devices = jax.devices()[:NCORES]
    mesh = Mesh(np.asarray(devices), ("core",))
    sharded = jax.jit(
        shard_map(_body, mesh=mesh,
                  in_specs=(PartitionSpec("core"),) * (n_params + n_outs),
                  out_specs=(PartitionSpec("core"),) * n_outs,
                  check_rep=False),
        donate_argnums=donate, keep_unused=True)

    per_core = [[np.asarray(m[name]) for name in in_names[:n_params]]
                for m in maps]
    concat_in = [np.concatenate([per_core[c][i] for c in range(NCORES)], axis=0)
                 for i in range(n_params)]
    concat_in = [jax.device_put(a) for a in concat_in]
    zeros = [np.zeros((NCORES * z.shape[0], *z.shape[1:]), z.dtype)
             for z in zero_outs]

    def run_once(prev):
        return sharded(*concat_in, *prev)

    # warmup (compiles + loads NEFF)
    outs = run_once(zeros)
    jax.block_until_ready(outs)
    return run_once, outs


def _steady(run_once, outs, iters):
    """Chained steady-state: feed previous outputs back as donated output
    buffers so everything stays device-resident. Returns s per dispatch."""
    import time
    import jax
    outs = run_once(outs)   # extra warm dispatch
    jax.block_until_ready(outs)
    t0 = time.time()
    for _ in range(iters):
        outs = run_once(outs)
    jax.block_until_ready(outs)
    return (time.time() - t0) / iters


VARIANT = "split_noyt"


def _build(k_loop=1):
    # split_noyt: two interleaved half-batch chains with the baseline's
    # py-group state path — HW numerics bit-identical to the baseline
    # (rel err 1.37e-3), ~11% faster than single-chain.
    if VARIANT == "split_noyt":
        return _build_nc_split(k_loop=k_loop, use_ysub=False)
    if VARIANT == "split":
        return _build_nc_split(k_loop=k_loop)
    return _build_nc(k_loop=k_loop)


def steady_state_time_ns(inputs, iters=25, k_big=257):
    """Measure HW execution time of one kernel pass.

    The axon/PJRT tunnel adds multiple ms of per-dispatch overhead that is
    not HW execution time (a trivial 1-copy NEFF measures ~3-5 ms/dispatch
    through the same path). To measure the kernel itself, the same NEFF body
    is compiled with a For_i hardware loop around it (k_big passes per
    dispatch, each pass a complete kernel execution: HBM x/weight reads,
    u-matmul, 64-step recurrence, readout, output DMA). Differencing against
    the single-pass NEFF cancels the per-dispatch overhead exactly:
        t_pass = (T(k_big) - T(1)) / (k_big - 1).
    Both are timed chained + device-resident over `iters` dispatches.
    """
    maps = _in_maps(np.asarray(inputs["x"]), np.asarray(inputs["W_fc"]),
                    np.asarray(inputs["b_fc"]), np.asarray(inputs["alpha"]),
                    np.asarray(inputs["beta"]), np.asarray(inputs["gamma"]),
                    np.asarray(inputs["W_out"]))
    nc1 = _build(k_loop=1)
    run1, outs1 = _make_runner(nc1, maps)
    ncb = _build(k_loop=k_big)
    runb, outsb = _make_runner(ncb, maps)

    # the k-looped NEFF must compute the exact same output every pass
    same = all(np.array_equal(np.asarray(a), np.asarray(b))
               for a, b in zip(outs1, outsb))
    print(f"  [timing detail] k-loop NEFF output identical to single-pass: "
          f"{same}")

    t1 = _steady(run1, outs1, iters)
    tb = _steady(runb, outsb, iters)
    t_pass = (tb - t1) / (k_big - 1)
    print(f"  [timing detail] T(K=1)={t1*1e3:.3f} ms/dispatch, "
          f"T(K={k_big})={tb*1e3:.3f} ms/dispatch, "
          f"amortized upper bound={tb/k_big*1e6:.1f} us/pass")
    return t_pass * 1e9


def kernel(x, W_fc, b_fc, alpha, beta, gamma, W_out, b_out, **kw):
    nc = _build()
    maps = _in_maps(np.asarray(x), np.asarray(W_fc), np.asarray(b_fc),
                    np.asarray(alpha), np.asarray(beta), np.asarray(gamma),
                    np.asarray(W_out))
    res = run_bass_kernel_spmd(nc, maps, core_ids=list(range(NCORES)))
    total = np.zeros((O, B), np.float64)
    for c in range(NCORES):
        total += res.results[c]["out"].astype(np.float64)
    total += np.asarray(b_out).astype(np.float64)[:, None]
    return np.ascontiguousarray(total.T).astype(np.float32)



# revision 34
# speedup vs baseline: 1.2122x; 1.2122x over previous
"""Trainium2 Bass kernel for the LiquidNeuralNetwork problem.

Math: h' = -alpha*h + beta*tanh(x_t @ W_fc.T + b_fc + gamma*h), piecewise-
constant input over 64 intervals; output = h(1.0) @ W_out.T + b_out.

Each hidden unit's ODE is independent given u = x@W_fc.T, so we shard the
HIDDEN dim across the 8 cores (128 units each, batch of 256 as the free
dim). Per core (variant "split_noyt", the default):
  1. U-matmul (fp16 operands, fp32 PSUM): u[h, (s,b)] for its hidden slice,
     trickled in 512-col blocks between recurrence steps so the PE never
     takes a long burst on the critical path.
  2. 64 sequential exponential-midpoint steps on the y = gamma*h state:
       y' = E*y + D*tanh(u + y + b_fc),  E = exp(-alpha*dt) etc.
     The batch is SPLIT INTO TWO INDEPENDENT 128-col CHAINS so the scalar
     engine's tanh of one half overlaps the tensor engine's matmuls of the
     other (the act->matmul->act dependency chain, not engine throughput, is
     the bottleneck). Per-unit scales are applied as diagonal matmuls
     accumulating in PSUM (fp32 for the state term - f32r is no faster at
     128 moving cols - fp16 for activation/input terms); tanh runs on the
     scalar engine reading PSUM directly with b_fc as per-partition bias.
     The state path (py = dE*y + dD*a2 -> DVE copy) is kept bit-identical
     to the proven single-chain baseline: the ODE is chaotic for high
     |gamma*beta| units (~1e3-1e4 amplification of state rounding), so
     alternative state arithmetic (e.g. y = pq - u) re-rolls the HW error
     draw from 1.4e-3 to ~9e-3 against the 2e-2 gate.
  3. Readout matmul against W_out.T / gamma (gamma folded in on host).
Host sums the 8 partial readouts and adds b_out.

Timing methodology: the axon/PJRT tunnel adds ms-scale per-dispatch overhead
(a trivial copy NEFF measures 3-15 ms/dispatch), so the whole kernel body is
also compiled wrapped in a For_i hardware loop (K complete passes per
dispatch, each re-reading x/weights from HBM and re-writing the output), and
HW exec time is the dispatch-time difference (T(K)-T(1))/(K-1), which
cancels dispatch overhead exactly. Measured: 166 us/pass vs 186 us for the
single-chain baseline and 13.1 ms for the per-dispatch wall-clock metric.
"""
import sys

sys.path.insert(0, "/opt/trn_rl_repo")

import numpy as np

import concourse.bacc as bacc
import concourse.mybir as mybir
import concourse.tile as tile
from concourse.bass_utils import run_bass_kernel_spmd

B, S, I, H, O = 256, 64, 256, 1024, 10
NCORES = 8
HS = H // NCORES          # hidden rows per core
DT = 1.0 / S
COLS = B                  # free dim of the recurrence state
NB = 512                  # matmul moving-dim block
CHUNK = 2048              # x/u column chunk (8 intervals worth)
NCHUNKS = S * B // CHUNK  # 8

F32 = mybir.dt.float32
BF16 = mybir.dt.bfloat16
F16 = mybir.dt.float16
F32R = mybir.dt.float32r
TANH = mybir.ActivationFunctionType.Tanh

_built = {}


def _build_nc_split(k_loop=1, use_ysub=True, sched=0):
    """Optimized per-core NEFF: the batch (256 cols) is split into two
    independent 128-col recurrence chains so the scalar engine's tanh of one
    half overlaps the tensor engine's matmuls of the other. The py group and
    state copy of the baseline are replaced by the exact identity
    y_{s+1} = pq_{s+1} - u_{s+1} (one DVE op), since
    pq_{s+1} = E*y_s + u_{s+1} + D*a2 and y_{s+1} = E*y_s + D*a2.
    """
    key = ("split", k_loop, use_ysub, sched)
    if key in _built:
        return _built[key]
    nc = bacc.Bacc("TRN2", target_bir_lowering=False, debug=False,
                   num_devices=NCORES)

    HB = COLS // 2  # 128-col half-batch per chain

    xk = [nc.dram_tensor(f"x{k}", [128, S * B], F16, kind="ExternalInput")
          for k in range(2)]
    wk = [nc.dram_tensor(f"w{k}", [128, HS], F16, kind="ExternalInput")
          for k in range(2)]
    dEh_d = nc.dram_tensor("dEh", [HS, HS], F32, kind="ExternalInput")
    dE_d = nc.dram_tensor("dE", [HS, HS], F32, kind="ExternalInput")
    dDh_d = nc.dram_tensor("dDh", [HS, HS], F16, kind="ExternalInput")
    dD_d = nc.dram_tensor("dD", [HS, HS], F16, kind="ExternalInput")
    dI_d = nc.dram_tensor("dI", [HS, HS], F16, kind="ExternalInput")
    bfc_d = nc.dram_tensor("bfc", [HS, 1], F32, kind="ExternalInput")
    wo_d = nc.dram_tensor("wo", [HS, O], F32, kind="ExternalInput")
    out_d = nc.dram_tensor("out", [O, B], F32, kind="ExternalOutput")

    with tile.TileContext(nc) as tc:
        with tc.tile_pool(name="const", bufs=1) as cpool, \
             tc.tile_pool(name="xpool", bufs=1) as xpool, \
             tc.tile_pool(name="upool", bufs=1) as upool, \
             tc.tile_pool(name="state", bufs=2) as spool, \
             tc.tile_pool(name="act", bufs=3) as apool, \
             tc.tile_pool(name="psmp", bufs=1, space="PSUM") as psmp, \
             tc.tile_pool(name="psqp", bufs=2, space="PSUM") as psqp, \
             tc.tile_pool(name="psu", bufs=2, space="PSUM") as psu:
            psm = [psmp, psmp]
            psq = [psqp, psqp]

            def body():
                wt = []
                for k in range(2):
                    t = cpool.tile([128, HS], F16, tag=f"w{k}", name=f"w{k}s")
                    nc.sync.dma_start(t[:], wk[k][:])
                    wt.append(t)
                dEh = cpool.tile([HS, HS], F32, tag="dEh")
                dE = cpool.tile([HS, HS], F32, tag="dE")
                dDh = cpool.tile([HS, HS], F16, tag="dDh")
                dD = cpool.tile([HS, HS], F16, tag="dD")
                dI = cpool.tile([HS, HS], F16, tag="dI")
                bfc = cpool.tile([HS, 1], F32, tag="bfc")
                wo = cpool.tile([HS, O], F32, tag="wo")
                for t, d in [(dDh, dDh_d), (dD, dD_d), (dI, dI_d),
                             (bfc, bfc_d), (wo, wo_d), (dEh, dEh_d),
                             (dE, dE_d)]:
                    nc.sync.dma_start(t[:], d[:])

                xt = [[None] * NCHUNKS for _ in range(2)]
                for k in range(2):
                    for c in range(NCHUNKS):
                        t = xpool.tile([128, CHUNK], F16, tag=f"x{k}_{c}",
                                       name=f"x{k}_{c}s")
                        nc.sync.dma_start(t[:], xk[k][:, c * CHUNK:(c + 1) * CHUNK])
                        xt[k][c] = t

                ut = [upool.tile([128, CHUNK], F16, tag=f"u{c}", name=f"u{c}s")
                      for c in range(NCHUNKS)]

                def emit_u_block(c, nb):
                    pu = psu.tile([128, NB], F32, tag="pu", name="pu")
                    sl = slice(nb * NB, (nb + 1) * NB)
                    nc.tensor.matmul(pu[:], wt[0][:], xt[0][c][:, sl],
                                     start=True, stop=False)
                    nc.tensor.matmul(pu[:], wt[1][:], xt[1][c][:, sl],
                                     start=False, stop=True)
                    nc.vector.tensor_copy(ut[c][:, sl], pu[:])

                def emit_u_chunk(c):
                    for nb in range(CHUNK // NB):
                        emit_u_block(c, nb)

                # u chunks 0,1 up front; later blocks trickled per step pair
                emit_u_chunk(0)
                emit_u_chunk(1)

                def u_sl(s, h):
                    c, off = divmod(s * COLS, CHUNK)
                    off += h * HB
                    return ut[c][:, off:off + HB]

                # per-half state + initial pq = u_0 (y=0)
                y = [None, None]
                pq = [None, None]
                a1 = [None, None]
                for h in range(2):
                    y[h] = spool.tile([HS, HB], F32, tag=f"y{h}",
                                      name=f"y{h}")
                    nc.vector.memset(y[h][:], 0.0)
                    pq[h] = psq[h].tile([HS, HB], F32, tag=f"pq{h}",
                                        name=f"pq{h}")
                    nc.tensor.matmul(pq[h][:], dI[:], u_sl(0, h),
                                     start=True, stop=True)

                for s in range(S):
                    # trickle u-matmul blocks for chunk c+2 (4 blocks over
                    # the 8 steps a chunk lasts -> one block every 2 steps)
                    c = (s * COLS) // CHUNK
                    phase = s % (CHUNK // COLS)
                    if phase % 2 == 0 and c + 2 < NCHUNKS:
                        emit_u_block(c + 2, phase // 2)

                    pm = [None, None]
                    if sched == 1:
                        # issue act-independent pm matmuls before the a1
                        # activations so PE overlaps ACT
                        for h in range(2):
                            pm[h] = psm[h].tile([HS, HB], F32, tag=f"pm{h}",
                                                name=f"pm{h}")
                            nc.tensor.matmul(pm[h][:], dEh[:], y[h][:],
                                             start=True, stop=False)
                            nc.tensor.matmul(pm[h][:], dI[:], u_sl(s, h),
                                             start=False, stop=False)
                    for h in range(2):
                        a1[h] = apool.tile([HS, HB], F16, tag=f"a1{h}",
                                           name=f"a1{h}")
                        nc.scalar.activation(a1[h][:], pq[h][:], TANH,
                                             bias=bfc[:], scale=1.0)

                    if sched == 0:
                        for h in range(2):
                            pm[h] = psm[h].tile([HS, HB], F32, tag=f"pm{h}",
                                                name=f"pm{h}")
                            nc.tensor.matmul(pm[h][:], dEh[:], y[h][:],
                                             start=True, stop=False)
                            nc.tensor.matmul(pm[h][:], dI[:], u_sl(s, h),
                                             start=False, stop=False)
                    for h in range(2):
                        nc.tensor.matmul(pm[h][:], dDh[:], a1[h][:],
                                         start=False, stop=True)

                    pqn = [None, None]

                    def emit_pq_state():
                        for h in range(2):
                            pqn[h] = psq[h].tile([HS, HB], F32, tag=f"pq{h}",
                                                 name=f"pq{h}")
                            nc.tensor.matmul(pqn[h][:], dE[:], y[h][:],
                                             start=True, stop=False)
                            if use_ysub != "acc":
                                nc.tensor.matmul(pqn[h][:], dI[:],
                                                 u_sl(s + 1, h),
                                                 start=False, stop=False)

                    if (sched == 1 or use_ysub == "acc") and s < S - 1:
                        # pq/py state matmuls run on PE during the a2 tanh
                        emit_pq_state()

                    a2 = [None, None]
                    for h in range(2):
                        a2[h] = apool.tile([HS, HB], F16, tag=f"a2{h}",
                                           name=f"a2{h}")
                        nc.scalar.activation(a2[h][:], pm[h][:], TANH,
                                             bias=bfc[:], scale=1.0)

                    if s < S - 1:
                        if sched == 0 and use_ysub != "acc":
                            emit_pq_state()
                        yn = [None, None]
                        if use_ysub == "acc":
                            # py = E*y + D*a2 (exact baseline state path);
                            # DVE-copy y out; then accumulate u' into the
                            # same bank so a1' reads (py + u') with no
                            # separate pq group
                            for h in range(2):
                                nc.tensor.matmul(pqn[h][:], dD[:], a2[h][:],
                                                 start=False, stop=True)
                            for h in range(2):
                                yn[h] = spool.tile([HS, HB], F32,
                                                   tag=f"y{h}", name=f"y{h}")
                                nc.vector.tensor_copy(yn[h][:], pqn[h][:])
                            for h in range(2):
                                nc.tensor.matmul(pqn[h][:], dI[:],
                                                 u_sl(s + 1, h),
                                                 start=False, stop=True)
                            pq = pqn
                            y = yn
                            continue
                        for h in range(2):
                            nc.tensor.matmul(pqn[h][:], dD[:], a2[h][:],
                                             start=False, stop=True)
                        if use_ysub:
                            # y_{s+1} = pq_{s+1} - u_{s+1}  (exact)
                            for h in range(2):
                                yn[h] = spool.tile([HS, HB], F32,
                                                   tag=f"y{h}", name=f"y{h}")
                                nc.vector.tensor_tensor(
                                    yn[h][:], pqn[h][:], u_sl(s + 1, h),
                                    mybir.AluOpType.subtract)
                        else:
                            # A/B check: baseline-style py group + copy
                            for h in range(2):
                                pyl = psm[h].tile([HS, HB], F32,
                                                  tag=f"pm{h}",
                                                  name=f"py{h}_{s}")
                                nc.tensor.matmul(pyl[:], dE[:], y[h][:],
                                                 start=True, stop=False)
                                nc.tensor.matmul(pyl[:], dD[:], a2[h][:],
                                                 start=False, stop=True)
                                yn[h] = spool.tile([HS, HB], F32,
                                                   tag=f"y{h}", name=f"y{h}")
                                nc.vector.tensor_copy(yn[h][:], pyl[:])
                        pq = pqn
                        y = yn
                    else:
                        # last step: y_S = E*y + D*a2 directly
                        pyl = [None, None]
                        for h in range(2):
                            pyl[h] = psm[h].tile([HS, HB], F32, tag=f"pm{h}",
                                                 name=f"py{h}")
                            nc.tensor.matmul(pyl[h][:], dE[:], y[h][:],
                                             start=True, stop=False)
                            nc.tensor.matmul(pyl[h][:], dD[:], a2[h][:],
                                             start=False, stop=True)
                        yn = [None, None]
                        for h in range(2):
                            yn[h] = spool.tile([HS, HB], F32, tag=f"y{h}",
                                               name=f"y{h}")
                            nc.vector.tensor_copy(yn[h][:], pyl[h][:])
                        y = yn

                # readout per half: out[o, b] = sum_p wo[p, o] * y[p, b]
                o32 = spool.tile([O, COLS], F32, tag="o32", name="o32")
                for h in range(2):
                    po = psm[h].tile([O, HB], F32, tag=f"pm{h}", name=f"po{h}")
                    nc.tensor.matmul(po[:], wo[:], y[h][:], start=True,
                                     stop=True)
                    nc.vector.tensor_copy(o32[:, h * HB:(h + 1) * HB], po[:])
                nc.sync.dma_start(out_d[:], o32[:])

            if k_loop == 1:
                body()
            else:
                with tc.For_i(0, k_loop):
                    body()

    nc.compile()
    _built[key] = nc
    return nc


def _build_nc(use_f32r=False, k_loop=1):
    """Build the per-core NEFF. k_loop>1 wraps the ENTIRE kernel (x DMA,
    u-matmul, recurrence, readout, output DMA) in a For_i hardware loop so
    steady-state HW time per pass can be measured without per-dispatch
    overhead."""
    key = (use_f32r, k_loop)
    if key in _built:
        return _built[key]
    nc = bacc.Bacc("TRN2", target_bir_lowering=False, debug=False,
                   num_devices=NCORES)

    # typed f32r: declare the fp32 state-matmul operands (dEh/dE/wo and the
    # recurrence state y) as float32r end-to-end so the BIR verifier sees
    # f32r-producing instructions feeding the f32r matmuls
    FSR = F32R if use_f32r else F32

    xk = [nc.dram_tensor(f"x{k}", [128, S * B], F16, kind="ExternalInput")
          for k in range(2)]
    wk = [nc.dram_tensor(f"w{k}", [128, HS], F16, kind="ExternalInput")
          for k in range(2)]
    dEh_d = nc.dram_tensor("dEh", [HS, HS], FSR, kind="ExternalInput")
    dE_d = nc.dram_tensor("dE", [HS, HS], FSR, kind="ExternalInput")
    dDh_d = nc.dram_tensor("dDh", [HS, HS], F16, kind="ExternalInput")
    dD_d = nc.dram_tensor("dD", [HS, HS], F16, kind="ExternalInput")
    dI_d = nc.dram_tensor("dI", [HS, HS], F16, kind="ExternalInput")
    bfc_d = nc.dram_tensor("bfc", [HS, 1], F32, kind="ExternalInput")
    wo_d = nc.dram_tensor("wo", [HS, O], FSR, kind="ExternalInput")
    out_d = nc.dram_tensor("out", [O, B], F32, kind="ExternalOutput")

    with tile.TileContext(nc) as tc:
        with tc.tile_pool(name="const", bufs=1) as cpool, \
             tc.tile_pool(name="xpool", bufs=1) as xpool, \
             tc.tile_pool(name="upool", bufs=1) as upool, \
             tc.tile_pool(name="state", bufs=3) as spool, \
             tc.tile_pool(name="act", bufs=3) as apool, \
             tc.tile_pool(name="psq", bufs=2, space="PSUM") as psq, \
             tc.tile_pool(name="psm", bufs=2, space="PSUM") as psm, \
             tc.tile_pool(name="psy", bufs=2, space="PSUM") as psy, \
             tc.tile_pool(name="psu", bufs=2, space="PSUM") as psu:

            def body():
                # ---- constants into SBUF ----
                wt = []
                for k in range(2):
                    t = cpool.tile([128, HS], F16, tag=f"w{k}", name=f"w{k}s")
                    nc.sync.dma_start(t[:], wk[k][:])
                    wt.append(t)
                dEh = cpool.tile([HS, HS], FSR, tag="dEh")
                dE = cpool.tile([HS, HS], FSR, tag="dE")
                dDh = cpool.tile([HS, HS], F16, tag="dDh")
                dD = cpool.tile([HS, HS], F16, tag="dD")
                dI = cpool.tile([HS, HS], F16, tag="dI")
                bfc = cpool.tile([HS, 1], F32, tag="bfc")
                wo = cpool.tile([HS, O], FSR, tag="wo")
                pairs = [(dDh, dDh_d), (dD, dD_d), (dI, dI_d), (bfc, bfc_d),
                         (wo, wo_d), (dEh, dEh_d), (dE, dE_d)]
                for t, d in pairs:
                    nc.sync.dma_start(t[:], d[:])

                # ---- x chunks ----
                xt = [[None] * NCHUNKS for _ in range(2)]
                for k in range(2):
                    for c in range(NCHUNKS):
                        t = xpool.tile([128, CHUNK], F16, tag=f"x{k}_{c}",
                                       name=f"x{k}_{c}s")
                        nc.sync.dma_start(t[:], xk[k][:, c * CHUNK:(c + 1) * CHUNK])
                        xt[k][c] = t

                ut = [upool.tile([128, CHUNK], F16, tag=f"u{c}", name=f"u{c}s")
                      for c in range(NCHUNKS)]

                def emit_u_chunk(c):
                    for nb in range(CHUNK // NB):
                        pu = psu.tile([128, NB], F32, tag="pu", name="pu")
                        sl = slice(nb * NB, (nb + 1) * NB)
                        nc.tensor.matmul(pu[:], wt[0][:], xt[0][c][:, sl],
                                         start=True, stop=False)
                        nc.tensor.matmul(pu[:], wt[1][:], xt[1][c][:, sl],
                                         start=False, stop=True)
                        nc.vector.tensor_copy(ut[c][:, sl], pu[:])

                def mmr(ps, lhs32, rhs32, start, stop):
                    # fp32 state-term matmul (operands FSR-typed: f32r full
                    # rate at >=256 moving cols, or plain fp32 at 1/4 rate)
                    nc.tensor.matmul(ps, lhs32, rhs32, start=start, stop=stop)

                # u chunks 0,1 up front; c+1 emitted while chunk c recurs
                emit_u_chunk(0)
                emit_u_chunk(1)

                y = spool.tile([HS, COLS], FSR, tag="y")
                if use_f32r:
                    # memset can't write f32r directly (ISA check); zero an
                    # fp32 scratch and convert via DVE copy
                    z32 = spool.tile([HS, COLS], F32, tag="z32", name="z32")
                    nc.vector.memset(z32[:], 0.0)
                    nc.vector.tensor_copy(y[:], z32[:])
                else:
                    nc.vector.memset(y[:], 0.0)
                pq = psq.tile([HS, COLS], F32, tag="pq", name="pq")
                nc.tensor.matmul(pq[:], dI[:], ut[0][:, 0:COLS],
                                 start=True, stop=True)

                for s in range(S):
                    c, off = divmod(s * COLS, CHUNK)
                    if s % (CHUNK // COLS) == 0 and c + 2 < NCHUNKS:
                        emit_u_chunk(c + 2)
                    u_s = ut[c][:, off:off + COLS]

                    a1 = apool.tile([HS, COLS], F16, tag="a1", name="a1")
                    nc.scalar.activation(a1[:], pq[:], TANH, bias=bfc[:],
                                         scale=1.0)

                    pm = psm.tile([HS, COLS], F32, tag="pm", name="pm")
                    mmr(pm[:], dEh[:], y[:], True, False)
                    nc.tensor.matmul(pm[:], dI[:], u_s, start=False, stop=False)
                    nc.tensor.matmul(pm[:], dDh[:], a1[:], start=False,
                                     stop=True)

                    a2 = apool.tile([HS, COLS], F16, tag="a2", name="a2")
                    nc.scalar.activation(a2[:], pm[:], TANH, bias=bfc[:],
                                         scale=1.0)

                    if s < S - 1:
                        cn, offn = divmod((s + 1) * COLS, CHUNK)
                        u_n = ut[cn][:, offn:offn + COLS]
                        pq = psq.tile([HS, COLS], F32, tag="pq", name="pq")
                        mmr(pq[:], dE[:], y[:], True, False)
                        nc.tensor.matmul(pq[:], dI[:], u_n, start=False,
                                         stop=False)
                        nc.tensor.matmul(pq[:], dD[:], a2[:], start=False,
                                         stop=True)

                    py = psy.tile([HS, COLS], F32, tag="py", name="py")
                    mmr(py[:], dE[:], y[:], True, False)
                    nc.tensor.matmul(py[:], dD[:], a2[:], start=False,
                                     stop=True)
                    y = spool.tile([HS, COLS], FSR, tag="y", name="y")
                    nc.vector.tensor_copy(y[:], py[:])

                # readout: out[o, b] = sum_p wo[p, o] * y64[p, b]
                po = psm.tile([O, COLS], F32, tag="pm", name="po")
                mmr(po[:], wo[:], y[:], True, True)
                o32 = spool.tile([O, COLS], F32, tag="o32", name="o32")
                nc.vector.tensor_copy(o32[:], po[:])
                nc.sync.dma_start(out_d[:], o32[:])

            if k_loop == 1:
                body()
            else:
                with tc.For_i(0, k_loop):
                    body()

    nc.compile()
    _built[key] = nc
    return nc


def _phi1(a, t):
    z = a * t
    small = np.abs(z) < 1e-6
    return np.where(small, 1 - z / 2 + z * z / 6,
                    (1 - np.exp(-z)) / np.where(small, 1, z))


def _in_maps(x, W_fc, b_fc, alpha, beta, gamma, W_out):
    a64 = alpha.astype(np.float64)
    b64 = beta.astype(np.float64)
    g64 = gamma.astype(np.float64)
    Eh = np.exp(-a64 * DT / 2)
    E = np.exp(-a64 * DT)
    Dh = g64 * b64 * (DT / 2) * _phi1(a64, DT / 2)
    D = g64 * b64 * DT * _phi1(a64, DT)

    # xT[i, s*B + b] = x[b, s, i]
    xT = np.ascontiguousarray(x.transpose(2, 1, 0).reshape(I, S * B))
    x16 = xT.astype(np.float16)
    eye16 = np.eye(HS, dtype=np.float16)

    g_safe = np.where(np.abs(g64) < 1e-30, 1e-30, g64)
    maps = []
    for c in range(NCORES):
        sl = slice(c * HS, (c + 1) * HS)
        wT = np.ascontiguousarray(W_fc[sl, :].T.astype(np.float16))  # [I, HS]
        maps.append({
            "x0": x16[:128], "x1": x16[128:],
            "w0": np.ascontiguousarray(wT[:128]),
            "w1": np.ascontiguousarray(wT[128:]),
            "dEh": np.ascontiguousarray(np.diag(Eh[sl]).astype(np.float32)),
            "dE": np.ascontiguousarray(np.diag(E[sl]).astype(np.float32)),
            "dDh": np.ascontiguousarray(np.diag(Dh[sl]).astype(np.float16)),
            "dD": np.ascontiguousarray(np.diag(D[sl]).astype(np.float16)),
            "dI": eye16,
            "bfc": b_fc[sl].astype(np.float32).reshape(HS, 1),
            "wo": np.ascontiguousarray(
                (W_out.astype(np.float64)[:, sl] / g_safe[sl][None, :])
                .T.astype(np.float32)),
        })
    return maps


def _make_runner(nc, maps):
    """Build a jitted sharded executor over the 8 cores for a compiled nc.
    Returns (run_once, outs0) where run_once(prev_outs) executes one dispatch
    with device-resident inputs, donating prev_outs as output buffers."""
    import jax
    from jax.sharding import Mesh, PartitionSpec
    from jax.experimental.shard_map import shard_map
    from concourse import bass2jax as b2j
    import concourse.mybir as mb

    b2j.install_neuronx_cc_hook()
    partition_name = (nc.partition_id_tensor.name
                      if nc.partition_id_tensor else None)
    in_names, out_names, out_avals, zero_outs = [], [], [], []
    for alloc in nc.m.functions[0].allocations:
        if not isinstance(alloc, mb.MemoryLocationSet):
            continue
        name = alloc.memorylocations[0].name
        if alloc.kind == "ExternalInput":
            if name != partition_name:
                in_names.append(name)
        elif alloc.kind == "ExternalOutput":
            shape = tuple(alloc.tensor_shape)
            dtype = mb.dt.np(alloc.dtype)
            out_avals.append(jax.core.ShapedArray(shape, dtype))
            zero_outs.append(np.zeros(shape, dtype))
            out_names.append(name)
    n_params = len(in_names)
    n_outs = len(out_avals)
    in_names.extend(out_names)
    if partition_name is not None:
        in_names.append(partition_name)

    donate = tuple(range(n_params, n_params + n_outs))

    def _body(*args):
        operands = list(args)
        if partition_name is not None:
            operands.append(b2j.partition_id_tensor())
        outs = b2j._bass_exec_p.bind(
            *operands, out_avals=tuple(out_avals), in_names=tuple(in_names),
            out_names=tuple(out_names), lowering_input_output_aliases=(),
            sim_require_finite=True, sim_require_nnan=True, nc=nc)
        return tuple(outs)

    devices = jax.devices()[:NCORES]
    mesh = Mesh(np.asarray(devices), ("core",))
    sharded = jax.jit(
        shard_map(_body, mesh=mesh,
                  in_specs=(PartitionSpec("core"),) * (n_params + n_outs),
                  out_specs=(PartitionSpec("core"),) * n_outs,
                  check_rep=False),
        donate_argnums=donate, keep_unused=True)

    per_core = [[np.asarray(m[name]) for name in in_names[:n_params]]
                for m in maps]
    concat_in = [np.concatenate([per_core[c][i] for c in range(NCORES)], axis=0)
                 for i in range(n_params)]
    concat_in = [jax.device_put(a) for a in concat_in]
    zeros = [np.zeros((NCORES * z.shape[0], *z.shape[1:]), z.dtype)
             for z in zero_outs]

    def run_once(prev):
        return sharded(*concat_in, *prev)

    # warmup (compiles + loads NEFF)
    outs = run_once(zeros)
    jax.block_until_ready(outs)
    return run_once, outs


def _steady(run_once, outs, iters):
    """Chained steady-state: feed previous outputs back as donated output
    buffers so everything stays device-resident. Returns (s per dispatch,
    live output buffers) — the passed-in buffers are donated/consumed."""
    import time
    import jax
    outs = run_once(outs)   # extra warm dispatch
    jax.block_until_ready(outs)
    t0 = time.time()
    for _ in range(iters):
        outs = run_once(outs)
    jax.block_until_ready(outs)
    return (time.time() - t0) / iters, outs


VARIANT = "split_noyt"


def _build(k_loop=1):
    # split_noyt: two interleaved half-batch chains with the baseline's
    # py-group state path — HW numerics bit-identical to the baseline
    # (rel err 1.37e-3), ~11% faster than single-chain.
    if VARIANT == "split_noyt":
        return _build_nc_split(k_loop=k_loop, use_ysub=False)
    if VARIANT == "split":
        return _build_nc_split(k_loop=k_loop)
    return _build_nc(k_loop=k_loop)


def steady_state_time_ns(inputs, iters=25, k_big=257):
    """Measure HW execution time of one kernel pass.

    The axon/PJRT tunnel adds multiple ms of per-dispatch overhead that is
    not HW execution time (a trivial 1-copy NEFF measures ~3-5 ms/dispatch
    through the same path). To measure the kernel itself, the same NEFF body
    is compiled with a For_i hardware loop around it (k_big passes per
    dispatch, each pass a complete kernel execution: HBM x/weight reads,
    u-matmul, 64-step recurrence, readout, output DMA). Differencing against
    the single-pass NEFF cancels the per-dispatch overhead exactly:
        t_pass = (T(k_big) - T(1)) / (k_big - 1).
    Both are timed chained + device-resident over `iters` dispatches.
    """
    maps = _in_maps(np.asarray(inputs["x"]), np.asarray(inputs["W_fc"]),
                    np.asarray(inputs["b_fc"]), np.asarray(inputs["alpha"]),
                    np.asarray(inputs["beta"]), np.asarray(inputs["gamma"]),
                    np.asarray(inputs["W_out"]))
    nc1 = _build(k_loop=1)
    run1, outs1 = _make_runner(nc1, maps)
    ncb = _build(k_loop=k_big)
    runb, outsb = _make_runner(ncb, maps)

    # the k-looped NEFF must compute the exact same output every pass
    same = all(np.array_equal(np.asarray(a), np.asarray(b))
               for a, b in zip(outs1, outsb))
    print(f"  [timing detail] k-loop NEFF output identical to single-pass: "
          f"{same}")

    # the shared device shows ~10-20% run-to-run variance; take the median
    # of three independent differential measurements
    samples = []
    for r in range(3):
        t1, outs1 = _steady(run1, outs1, iters)
        tb, outsb = _steady(runb, outsb, iters)
        samples.append((tb - t1) / (k_big - 1))
        print(f"  [timing detail] rep{r}: T(K=1)={t1*1e3:.3f} ms, "
              f"T(K={k_big})={tb*1e3:.3f} ms, "
              f"diff={(tb - t1) / (k_big - 1)*1e6:.1f} us/pass, "
              f"upper bound={tb/k_big*1e6:.1f} us/pass")
    t_pass = sorted(samples)[1]
    return t_pass * 1e9


def kernel(x, W_fc, b_fc, alpha, beta, gamma, W_out, b_out, **kw):
    nc = _build()
    maps = _in_maps(np.asarray(x), np.asarray(W_fc), np.asarray(b_fc),
                    np.asarray(alpha), np.asarray(beta), np.asarray(gamma),
                    np.asarray(W_out))
    res = run_bass_kernel_spmd(nc, maps, core_ids=list(range(NCORES)))
    total = np.zeros((O, B), np.float64)
    for c in range(NCORES):
        total += res.results[c]["out"].astype(np.float64)
    total += np.asarray(b_out).astype(np.float64)[:, None]
    return np.ascontiguousarray(total.T).astype(np.float32)

